# revision 7
# baseline (speedup 1.0000x reference)
"""Trainium2 Bass kernel for nn_InverseDCT (8x8 block IDCT + de-standardize
+ pixel interleave) — v2.

Math:
  out[b, 0, 8h+x, 8w+y] = (sum_{u,v} M[(x,y),(u,v)] * (dct[b,(u,v),h,w]*std + mean)
                           + 128) / 255
with M[(x,y),(u,v)] = scale[u,v]*basis[x,y,u,v] (64x64 constant).  std/255 is
folded into M on the host; ((M@mean)+128)/255 is a scalar bias when mean == 0
(the spec'd case); otherwise mean/std is folded into the data on the host.

Key differences vs v1:
  * Input is cast to bf16 on the host: halves input HBM traffic and removes
    the on-device GpSimd cast (was 7us/strip).
  * Input DMA access pattern puts uv (64) as the OUTERMOST DRAM dim.  HWDGE
    assigns descriptors to SDMA engines by outer-dim index; the v1 pattern
    (outer dim = 2) serialized the whole input stream onto 2 of 16 engines
    (~26 GB/s each — measured as 645us of the 752us runtime).
  * The IDCT matrix's column order is (y, s, x) so that the PSUM->SBUF copy
    after the matmul has 8-element contiguous runs on BOTH sides (v1 wrote
    stride-512B single elements — 2.7us per 512-elem instruction on ACT).
  * Transposes write bf16 PSUM; the final y-interleave copy runs on DVE.

Per-core dataflow (pure data parallel over batch, 2 batches / core), one
"strip" = 16 block-rows = 128 output image rows:
    1. SP     DMA-in  X[128, 2048] bf16; partitions p = uv*2 + s
    2. PE     16 matmuls: lhsT = X chunk (dh,wsel) [128x128] (stationary),
              rhs = MTs (block-diag IDCT, cols ordered (y,s,x))
              -> P[wsel][p=w'', f=(dh,(y,s,x))] f32
    3. Act    4 copies (wsel,s): P -> S2[p=w'', f=(wsel,y,(s,dh,x))] bf16
              with +bias; 8-contig runs both sides
    4. PE     16 transposes (wsel,y): S2 chunk -> Q[p=row=(s,dh,x), f=w'']
    5. DVE    2 copies (wsel): Q -> R[p=row, f=wsel*1024 + w''*8 + y] f32
    6. Act    DMA-out R[128, 2048] f32 -> contiguous 1MB image-row block
"""

import os
import sys

import numpy as np

for _p in ("/opt/trn_rl_repo",):
    if _p not in sys.path and os.path.isdir(_p):
        sys.path.append(_p)

N_CORES = 8
B_FULL = 16
B_PC = B_FULL // N_CORES  # batches per core
C = 64
H = W = 256
STRIPS_PER_BATCH = 16  # 16 block-rows each -> 2048 rows
N_STRIPS = B_PC * STRIPS_PER_BATCH  # 32


def _idct_matrix():
    # mirror reference._idct_tables in float64
    steps = np.arange(8, dtype=np.float64) / 16.0
    f = 2.0 * np.arange(8, dtype=np.float64) + 1.0
    h = np.cos(np.outer(steps, f * np.pi))  # [u, x]
    basis = h.T[:, None, :, None] * h.T[None, :, None, :]  # [x, y, u, v]
    c = np.ones(8, dtype=np.float64)
    c[0] = np.sqrt(0.5)
    scale = 0.25 * np.outer(c, c)  # [u, v]
    M = (scale[None, None, :, :] * basis).reshape(64, 64)  # [(x,y), (u,v)]
    return M


def _build_nc(bias_scalar: float, repeat: int = 1):
    import concourse.bass as bass
    import concourse.mybir as mybir

    nc = bass.Bass()
    f32 = mybir.dt.float32
    bf16 = mybir.dt.bfloat16

    dct_in = nc.dram_tensor("dct", [B_PC, C, H, W], bf16, kind="ExternalInput")
    mi_in = nc.dram_tensor("mi", [128, 256], bf16, kind="ExternalInput")
    bias_in = nc.dram_tensor("biasv", [128, 1], f32, kind="ExternalInput")
    out = nc.dram_tensor("out", [B_PC, 8 * H, 8 * W], bf16, kind="ExternalOutput")

    IDENT_FUNC = mybir.ActivationFunctionType.Identity

    from contextlib import ExitStack

    NS = N_STRIPS * repeat

    with ExitStack() as stack:
        xa = stack.enter_context(nc.sbuf_tensor("xa", [128, 2048], bf16))
        xb = stack.enter_context(nc.sbuf_tensor("xb", [128, 2048], bf16))
        xc = stack.enter_context(nc.sbuf_tensor("xc", [128, 2048], bf16))
        xd = stack.enter_context(nc.sbuf_tensor("xd", [128, 2048], bf16))
        xe = stack.enter_context(nc.sbuf_tensor("xe", [128, 2048], bf16))
        xf = stack.enter_context(nc.sbuf_tensor("xf", [128, 2048], bf16))
        s2a = stack.enter_context(nc.sbuf_tensor("s2a", [128, 2048], bf16))
        s2b = stack.enter_context(nc.sbuf_tensor("s2b", [128, 2048], bf16))
        s2c = stack.enter_context(nc.sbuf_tensor("s2c", [128, 2048], bf16))
        ra = stack.enter_context(nc.sbuf_tensor("ra", [128, 2048], bf16))
        rb = stack.enter_context(nc.sbuf_tensor("rb", [128, 2048], bf16))
        rc = stack.enter_context(nc.sbuf_tensor("rc", [128, 2048], bf16))
        rd = stack.enter_context(nc.sbuf_tensor("rd", [128, 2048], bf16))
        mi_sb = stack.enter_context(nc.sbuf_tensor("mi_sb", [128, 256], bf16))
        bias_sb = stack.enter_context(nc.sbuf_tensor("bias_sb", [128, 1], f32))
        p0 = stack.enter_context(nc.psum_tensor("p0", [128, 1024], f32))
        p1 = stack.enter_context(nc.psum_tensor("p1", [128, 1024], f32))
        q0 = stack.enter_context(nc.psum_tensor("q0", [128, 2048], bf16))
        q1 = stack.enter_context(nc.psum_tensor("q1", [128, 2048], bf16))
        s_cst = stack.enter_context(nc.semaphore("s_cst"))
        s_in0 = stack.enter_context(nc.semaphore("s_in0"))
        s_in1 = stack.enter_context(nc.semaphore("s_in1"))
        s_in2 = stack.enter_context(nc.semaphore("s_in2"))
        s_in3 = stack.enter_context(nc.semaphore("s_in3"))
        s_in4 = stack.enter_context(nc.semaphore("s_in4"))
        s_in5 = stack.enter_context(nc.semaphore("s_in5"))
        s_mm = stack.enter_context(nc.semaphore("s_mm"))
        s_ca = stack.enter_context(nc.semaphore("s_ca"))
        s_t2 = stack.enter_context(nc.semaphore("s_t2"))
        s_cb = stack.enter_context(nc.semaphore("s_cb"))
        s_od = stack.enter_context(nc.semaphore("s_od"))
        block = stack.enter_context(nc.Block())
        X = [xa, xb, xc, xd, xe, xf]
        S_IN = [s_in0, s_in1, s_in2, s_in3, s_in4, s_in5]
        S2 = [s2a, s2b, s2c]
        R = [ra, rb, rc, rd]
        P = [p0, p1]
        Q = [q0, q1]  # Q[st%2]

        def dct_ap(st):
            # DRAM access pattern for one strip of input (512KB bf16).
            # OUTER dim is uv (64) so descriptors spread over all 16 SDMA
            # engines (engine = outer-dim index mod 16).
            # partitions p = uv*2 + s ; free = (dh_lo 8, w 256) contiguous 4KB
            st = st % N_STRIPS
            b = st // STRIPS_PER_BATCH
            stg = st % STRIPS_PER_BATCH
            base = b * (C * H * W) + stg * 16 * W
            return bass.AP(
                dct_in,
                base,
                [
                    [H * W, 64],  # uv: channel planes (outer -> engine spread)
                    [8 * W, 2],  # s: +8 block-rows
                    [1, 8 * W],  # (dh_lo, w) contiguous 4KB
                ],
            )

        def out_ap(st):
            # One strip of output rows: a single contiguous 1MB DRAM block.
            st = st % N_STRIPS
            b = st // STRIPS_PER_BATCH
            stg = st % STRIPS_PER_BATCH
            base = b * (8 * H * 8 * W) + stg * 128 * 2048
            return bass.AP(out, base, [[2048, 128], [1, 2048]])

        @block.sync
        def _(sync):
            # const loads first: their ~2us completion receipt is what gates
            # the first matmul, and merging mts+ident halves the issue count
            sync.dma_start(mi_sb[:, :], mi_in[:, :]).then_inc(s_cst, 16)
            sync.dma_start(bias_sb[:, :], bias_in[:, :]).then_inc(s_cst, 16)
            for st in range(NS):
                if st >= 6:
                    # X buffer reusable once all matmuls of strip st-6 done
                    sync.wait_ge(s_mm, (st - 5) * 16)
                sync.dma_start(X[st % 6][:, :], dct_ap(st)).then_inc(S_IN[st % 6], 16)

        @block.tensor
        def _(tensor):
            # HAM warm-up: ~32 dummy matmuls on garbage SBUF while waiting for
            # the first input DMA (PE idle 7-11.5us otherwise).  Fills the
            # free-running 4096-cycle activity window so the clock gate is at
            # 8/8 (2.4 GHz) when real work starts.  Outputs land in p0, which
            # every real matmul overwrites with start=True before any reader.
            for _ in range(32):
                tensor.matmul(
                    p0[:, 0:128],
                    s2a[:, 0:128],
                    s2a[:, 128:256],
                )
            tensor.wait_ge(s_cst, 32)

            def emit_mm(st, wsel):
                ph = st * 2 + wsel
                if ph >= 2:
                    # P[ph%2] reusable once copy-a(ph-2) done
                    tensor.wait_ge(s_ca, ph - 1)
                for dh in range(8):
                    col0 = dh * 256 + wsel * 128
                    tensor.matmul(
                        P[ph % 2][:, dh * 128 : (dh + 1) * 128],
                        X[st % 6][:, col0 : col0 + 128],
                        mi_sb[:, 0:128],
                    ).then_inc(s_mm, 1)

            def emit_t2(st, wsel):
                # transposes for strip st: S2[st%3] chunk (wsel,y) -> Q
                qpar = st % 2
                par = st % 3
                tensor.wait_ge(s_ca, st * 2 + wsel + 1)
                if wsel == 0 and st >= 2:
                    # Q[qpar] reusable once copy-b(st-2) done
                    tensor.wait_ge(s_cb, st - 1)
                for y in range(8):
                    tensor.matmul(
                        Q[qpar][:, wsel * 1024 + y * 128 : wsel * 1024 + (y + 1) * 128],
                        S2[par][:, wsel * 1024 + y * 128 : wsel * 1024 + (y + 1) * 128],
                        mi_sb[:, 128:256],
                        is_transpose=True,
                    ).then_inc(s_t2, 1)

            for st in range(NS):
                tensor.wait_ge(S_IN[st % 6], (st // 6 + 1) * 16)
                emit_mm(st, 0)
                if st >= 1:
                    emit_t2(st - 1, 0)
                emit_mm(st, 1)
                if st >= 1:
                    emit_t2(st - 1, 1)
            emit_t2(NS - 1, 0)
            emit_t2(NS - 1, 1)

        @block.scalar
        def _(scalar):
            scalar.wait_ge(s_cst, 32)
            for st in range(NS):
                for wsel in range(2):
                    ph = st * 2 + wsel
                    scalar.wait_ge(s_mm, st * 16 + (wsel + 1) * 8)
                    if st >= 3:
                        # S2[st%3] wsel-half reusable once tr(st-3, wsel) done
                        scalar.wait_ge(s_t2, (st - 3) * 16 + (wsel + 1) * 8)
                    # read P[ph%2][p, f = dh*128 + (ys)*8 + x]  (ys = y*2+s)
                    # write S2[p, f = wsel*1024 + (ys)*64 + dh*8 + x]
                    # (y,s) merge into one AP dim on both sides -> 3-dim APs,
                    # one instruction per wsel-half, 8-contig runs both sides
                    in_ap = bass.AP(
                        P[ph % 2],
                        0,
                        [[1024, 128], [8, 16], [128, 8], [1, 8]],  # part, ys, dh, x
                    )
                    out_ap_ = bass.AP(
                        S2[st % 3],
                        wsel * 1024,
                        [[2048, 128], [64, 16], [8, 8], [1, 8]],  # part, ys, dh, x
                    )
                    scalar.activation(
                        out_ap_, in_ap, IDENT_FUNC, bias=bias_sb[:, :], scale=1.0
                    ).then_inc(s_ca, 1)


        @block.vector
        def _(vector):
            for st in range(NS):
                par = st % 2
                if st < NS - 1:
                    vector.wait_ge(s_t2, (st + 1) * 16)
                    if st >= 4:
                        # R[st%4] reusable once out-DMA of strip st-4 done
                        vector.wait_ge(s_od, (st - 3) * 16)
                    # read Q[par][p, f = wsel*1024 + y*128 + w'']
                    # write R[p, f = wsel*1024 + w''*8 + y]
                    in_ap = bass.AP(
                        Q[par],
                        0,
                        [[2048, 128], [1024, 2], [1, 128], [128, 8]],
                    )
                    out_ap_ = bass.AP(
                        R[st % 4],
                        0,
                        [[2048, 128], [1024, 2], [8, 128], [1, 8]],
                    )
                    vector.tensor_copy(out_ap_, in_ap).then_inc(s_cb, 1)
                else:
                    # last strip: split by wsel so the final out-DMA halves
                    # can start as soon as each half is interleaved
                    if st >= 4:
                        vector.wait_ge(s_od, (st - 3) * 16)
                    for wsel in range(2):
                        vector.wait_ge(s_t2, st * 16 + (wsel + 1) * 8)
                        in_ap = bass.AP(
                            Q[par],
                            wsel * 1024,
                            [[2048, 128], [1, 128], [128, 8]],  # part, w'', y
                        )
                        out_ap_ = bass.AP(
                            R[st % 4],
                            wsel * 1024,
                            [[2048, 128], [8, 128], [1, 8]],  # part, w'', y
                        )
                        vector.tensor_copy(out_ap_, in_ap).then_inc(s_cb, 1)

        @block.gpsimd
        def _(pool):
            # out-DMAs on the SWDGE path: keeps the ACT engine free for the
            # PSUM->SBUF copies; descriptors spread via the partition swizzle.
            for st in range(NS - 1):
                pool.wait_ge(s_cb, st + 1)
                pool.dma_start(out_ap(st), R[st % 4][:, :]).then_inc(s_od, 16)
            # last strip in halves (cb increments s_cb twice for it)
            st = NS - 1
            b = (st % N_STRIPS) // STRIPS_PER_BATCH
            stg = (st % N_STRIPS) % STRIPS_PER_BATCH
            base = b * (8 * H * 8 * W) + stg * 128 * 2048
            for wsel in range(2):
                pool.wait_ge(s_cb, NS + wsel)
                pool.dma_start(
                    bass.AP(out, base + wsel * 1024, [[2048, 128], [1, 1024]]),
                    R[st % 4][:, wsel * 1024 : (wsel + 1) * 1024],
                ).then_inc(s_od, 16)

    return nc


def _host_prep(dct: np.ndarray, mean: np.ndarray, std: np.ndarray):
    import ml_dtypes

    dct = np.asarray(dct, dtype=np.float32)
    mean = np.asarray(mean, dtype=np.float64)
    std = np.asarray(std, dtype=np.float64)

    M = _idct_matrix()  # [(x,y), (u,v)]
    bias_vec = (M @ mean + 128.0) / 255.0  # [(x,y)]
    if np.ptp(bias_vec) > 1e-12:
        # General-mean fallback: fold the channel means into the data on the
        # host (never triggers for the spec'd inputs where mean == 0).
        safe_std = np.where(std == 0.0, 1.0, std)
        dct = dct + (mean / safe_std)[None, :, None, None].astype(np.float32)
        bias_scalar = float(128.0 / 255.0)
    else:
        bias_scalar = float(bias_vec[0])

    MT = M.T * std[:, None] / 255.0  # [uv, xy] with xy = x*8+y
    # MTs128[uv*2 + s, y*16 + s*8 + x] = MT[uv, x*8+y]
    MTs = np.zeros((128, 128), dtype=np.float64)
    uv = np.arange(64)
    for s in range(2):
        rows = uv * 2 + s  # partition order (uv, s)
        for y in range(8):
            for x in range(8):
                MTs[rows, y * 16 + s * 8 + x] = MT[uv, x * 8 + y]
    MTs_bf = MTs.astype(ml_dtypes.bfloat16)
    ident_bf = np.eye(128, dtype=np.float64).astype(ml_dtypes.bfloat16)

    dct_bf = dct.astype(ml_dtypes.bfloat16)

    in_maps = []
    for i in range(N_CORES):
        in_maps.append(
            {
                "dct": np.ascontiguousarray(dct_bf[i * B_PC : (i + 1) * B_PC]),
                "mi": np.concatenate([MTs_bf, ident_bf], axis=1),
                "biasv": np.full((128, 1), bias_scalar, dtype=np.float32),
            }
        )
    return in_maps, bias_scalar


def kernel(dct: np.ndarray, mean: np.ndarray, std: np.ndarray) -> np.ndarray:
    from concourse.bass_utils import run_bass_kernel_spmd

    in_maps, bias_scalar = _host_prep(dct, mean, std)
    nc = _build_nc(bias_scalar)

    res = run_bass_kernel_spmd(nc, in_maps, list(range(N_CORES)))

    full = np.empty((B_FULL, 1, 8 * H, 8 * W), dtype=np.float32)
    for i in range(N_CORES):
        full[i * B_PC : (i + 1) * B_PC, 0] = np.asarray(res.results[i]["out"]).astype(np.float32)
    return full


# revision 8
# speedup vs baseline: 1.0193x; 1.0193x over previous
"""Trainium2 Bass kernel for nn_InverseDCT (8x8 block IDCT + de-standardize
+ pixel interleave) — v2.

Math:
  out[b, 0, 8h+x, 8w+y] = (sum_{u,v} M[(x,y),(u,v)] * (dct[b,(u,v),h,w]*std + mean)
                           + 128) / 255
with M[(x,y),(u,v)] = scale[u,v]*basis[x,y,u,v] (64x64 constant).  std/255 is
folded into M on the host; ((M@mean)+128)/255 is a scalar bias when mean == 0
(the spec'd case); otherwise mean/std is folded into the data on the host.

Key differences vs v1:
  * Input is cast to bf16 on the host: halves input HBM traffic and removes
    the on-device GpSimd cast (was 7us/strip).
  * Input DMA access pattern puts uv (64) as the OUTERMOST DRAM dim.  HWDGE
    assigns descriptors to SDMA engines by outer-dim index; the v1 pattern
    (outer dim = 2) serialized the whole input stream onto 2 of 16 engines
    (~26 GB/s each — measured as 645us of the 752us runtime).
  * The IDCT matrix's column order is (y, s, x) so that the PSUM->SBUF copy
    after the matmul has 8-element contiguous runs on BOTH sides (v1 wrote
    stride-512B single elements — 2.7us per 512-elem instruction on ACT).
  * Transposes write bf16 PSUM; the final y-interleave copy runs on DVE.

Per-core dataflow (pure data parallel over batch, 2 batches / core), one
"strip" = 16 block-rows = 128 output image rows:
    1. SP     DMA-in  X[128, 2048] bf16; partitions p = uv*2 + s
    2. PE     16 matmuls: lhsT = X chunk (dh,wsel) [128x128] (stationary),
              rhs = MTs (block-diag IDCT, cols ordered (y,s,x))
              -> P[wsel][p=w'', f=(dh,(y,s,x))] f32
    3. Act    4 copies (wsel,s): P -> S2[p=w'', f=(wsel,y,(s,dh,x))] bf16
              with +bias; 8-contig runs both sides
    4. PE     16 transposes (wsel,y): S2 chunk -> Q[p=row=(s,dh,x), f=w'']
    5. DVE    2 copies (wsel): Q -> R[p=row, f=wsel*1024 + w''*8 + y] f32
    6. Act    DMA-out R[128, 2048] f32 -> contiguous 1MB image-row block
"""

import os
import sys

import numpy as np

for _p in ("/opt/trn_rl_repo",):
    if _p not in sys.path and os.path.isdir(_p):
        sys.path.append(_p)

N_CORES = 8
B_FULL = 16
B_PC = B_FULL // N_CORES  # batches per core
C = 64
H = W = 256
STRIPS_PER_BATCH = 16  # 16 block-rows each -> 2048 rows
N_STRIPS = B_PC * STRIPS_PER_BATCH  # 32


def _idct_matrix():
    # mirror reference._idct_tables in float64
    steps = np.arange(8, dtype=np.float64) / 16.0
    f = 2.0 * np.arange(8, dtype=np.float64) + 1.0
    h = np.cos(np.outer(steps, f * np.pi))  # [u, x]
    basis = h.T[:, None, :, None] * h.T[None, :, None, :]  # [x, y, u, v]
    c = np.ones(8, dtype=np.float64)
    c[0] = np.sqrt(0.5)
    scale = 0.25 * np.outer(c, c)  # [u, v]
    M = (scale[None, None, :, :] * basis).reshape(64, 64)  # [(x,y), (u,v)]
    return M


def _build_nc(bias_scalar: float, repeat: int = 1):
    import concourse.bass as bass
    import concourse.mybir as mybir

    nc = bass.Bass()
    f32 = mybir.dt.float32
    bf16 = mybir.dt.bfloat16

    dct_in = nc.dram_tensor("dct", [B_PC, C, H, W], bf16, kind="ExternalInput")
    mi_in = nc.dram_tensor("mi", [128, 256], bf16, kind="ExternalInput")
    bias_in = nc.dram_tensor("biasv", [128, 1], f32, kind="ExternalInput")
    out = nc.dram_tensor("out", [B_PC, 8 * H, 8 * W], bf16, kind="ExternalOutput")

    IDENT_FUNC = mybir.ActivationFunctionType.Identity

    from contextlib import ExitStack

    NS = N_STRIPS * repeat

    with ExitStack() as stack:
        xa = stack.enter_context(nc.sbuf_tensor("xa", [128, 2048], bf16))
        xb = stack.enter_context(nc.sbuf_tensor("xb", [128, 2048], bf16))
        xc = stack.enter_context(nc.sbuf_tensor("xc", [128, 2048], bf16))
        xd = stack.enter_context(nc.sbuf_tensor("xd", [128, 2048], bf16))
        xe = stack.enter_context(nc.sbuf_tensor("xe", [128, 2048], bf16))
        xf = stack.enter_context(nc.sbuf_tensor("xf", [128, 2048], bf16))
        s2a = stack.enter_context(nc.sbuf_tensor("s2a", [128, 2048], bf16))
        s2b = stack.enter_context(nc.sbuf_tensor("s2b", [128, 2048], bf16))
        s2c = stack.enter_context(nc.sbuf_tensor("s2c", [128, 2048], bf16))
        ra = stack.enter_context(nc.sbuf_tensor("ra", [128, 2048], bf16))
        rb = stack.enter_context(nc.sbuf_tensor("rb", [128, 2048], bf16))
        rc = stack.enter_context(nc.sbuf_tensor("rc", [128, 2048], bf16))
        rd = stack.enter_context(nc.sbuf_tensor("rd", [128, 2048], bf16))
        mi_sb = stack.enter_context(nc.sbuf_tensor("mi_sb", [128, 256], bf16))
        bias_sb = stack.enter_context(nc.sbuf_tensor("bias_sb", [128, 1], f32))
        p0 = stack.enter_context(nc.psum_tensor("p0", [128, 1024], f32))
        p1 = stack.enter_context(nc.psum_tensor("p1", [128, 1024], f32))
        q0 = stack.enter_context(nc.psum_tensor("q0", [128, 2048], bf16))
        q1 = stack.enter_context(nc.psum_tensor("q1", [128, 2048], bf16))
        s_cst = stack.enter_context(nc.semaphore("s_cst"))
        s_in0 = stack.enter_context(nc.semaphore("s_in0"))
        s_in1 = stack.enter_context(nc.semaphore("s_in1"))
        s_in2 = stack.enter_context(nc.semaphore("s_in2"))
        s_in3 = stack.enter_context(nc.semaphore("s_in3"))
        s_in4 = stack.enter_context(nc.semaphore("s_in4"))
        s_in5 = stack.enter_context(nc.semaphore("s_in5"))
        s_mm = stack.enter_context(nc.semaphore("s_mm"))
        s_ca = stack.enter_context(nc.semaphore("s_ca"))
        s_t2 = stack.enter_context(nc.semaphore("s_t2"))
        s_cb = stack.enter_context(nc.semaphore("s_cb"))
        s_od = stack.enter_context(nc.semaphore("s_od"))
        block = stack.enter_context(nc.Block())
        X = [xa, xb, xc, xd, xe, xf]
        S_IN = [s_in0, s_in1, s_in2, s_in3, s_in4, s_in5]
        S2 = [s2a, s2b, s2c]
        R = [ra, rb, rc, rd]
        P = [p0, p1]
        Q = [q0, q1]  # Q[st%2]

        def dct_ap(st):
            # DRAM access pattern for one strip of input (512KB bf16).
            # OUTER dim is uv (64) so descriptors spread over all 16 SDMA
            # engines (engine = outer-dim index mod 16).
            # partitions p = uv*2 + s ; free = (dh_lo 8, w 256) contiguous 4KB
            st = st % N_STRIPS
            b = st // STRIPS_PER_BATCH
            stg = st % STRIPS_PER_BATCH
            base = b * (C * H * W) + stg * 16 * W
            return bass.AP(
                dct_in,
                base,
                [
                    [H * W, 64],  # uv: channel planes (outer -> engine spread)
                    [8 * W, 2],  # s: +8 block-rows
                    [1, 8 * W],  # (dh_lo, w) contiguous 4KB
                ],
            )

        def out_ap(st):
            # One strip of output rows: a single contiguous 1MB DRAM block.
            st = st % N_STRIPS
            b = st // STRIPS_PER_BATCH
            stg = st % STRIPS_PER_BATCH
            base = b * (8 * H * 8 * W) + stg * 128 * 2048
            return bass.AP(out, base, [[2048, 128], [1, 2048]])

        @block.sync
        def _(sync):
            # const loads first: their ~2us completion receipt is what gates
            # the first matmul, and merging mts+ident halves the issue count
            sync.dma_start(mi_sb[:, :], mi_in[:, :]).then_inc(s_cst, 16)
            sync.dma_start(bias_sb[:, :], bias_in[:, :]).then_inc(s_cst, 16)
            for st in range(NS):
                if st >= 6:
                    # X buffer reusable once all matmuls of strip st-6 done
                    sync.wait_ge(s_mm, (st - 5) * 16)
                sync.dma_start(X[st % 6][:, :], dct_ap(st)).then_inc(S_IN[st % 6], 16)

        @block.tensor
        def _(tensor):
            # HAM warm-up: ~32 dummy matmuls on garbage SBUF while waiting for
            # the first input DMA (PE idle 7-11.5us otherwise).  Fills the
            # free-running 4096-cycle activity window so the clock gate is at
            # 8/8 (2.4 GHz) when real work starts.  Outputs land in p0, which
            # every real matmul overwrites with start=True before any reader.
            for _ in range(40):
                tensor.matmul(
                    p0[:, 0:128],
                    s2a[:, 0:128],
                    s2a[:, 128:256],
                )
            tensor.wait_ge(s_cst, 32)

            def emit_mm(st, wsel):
                ph = st * 2 + wsel
                if ph >= 2:
                    # P[ph%2] reusable once copy-a(ph-2) done
                    tensor.wait_ge(s_ca, ph - 1)
                for dh in range(8):
                    col0 = dh * 256 + wsel * 128
                    tensor.matmul(
                        P[ph % 2][:, dh * 128 : (dh + 1) * 128],
                        X[st % 6][:, col0 : col0 + 128],
                        mi_sb[:, 0:128],
                    ).then_inc(s_mm, 1)

            def emit_t2(st, wsel):
                # transposes for strip st: S2[st%3] chunk (wsel,y) -> Q
                qpar = st % 2
                par = st % 3
                tensor.wait_ge(s_ca, st * 2 + wsel + 1)
                if wsel == 0 and st >= 2:
                    # Q[qpar] reusable once copy-b(st-2) done
                    tensor.wait_ge(s_cb, st - 1)
                for y in range(8):
                    tensor.matmul(
                        Q[qpar][:, wsel * 1024 + y * 128 : wsel * 1024 + (y + 1) * 128],
                        S2[par][:, wsel * 1024 + y * 128 : wsel * 1024 + (y + 1) * 128],
                        mi_sb[:, 128:256],
                        is_transpose=True,
                    ).then_inc(s_t2, 1)

            for st in range(NS):
                tensor.wait_ge(S_IN[st % 6], (st // 6 + 1) * 16)
                emit_mm(st, 0)
                if st >= 1:
                    emit_t2(st - 1, 0)
                emit_mm(st, 1)
                if st >= 1:
                    emit_t2(st - 1, 1)
            emit_t2(NS - 1, 0)
            emit_t2(NS - 1, 1)

        @block.scalar
        def _(scalar):
            scalar.wait_ge(s_cst, 32)
            for st in range(NS):
                for wsel in range(2):
                    ph = st * 2 + wsel
                    scalar.wait_ge(s_mm, st * 16 + (wsel + 1) * 8)
                    if st >= 3:
                        # S2[st%3] wsel-half reusable once tr(st-3, wsel) done
                        scalar.wait_ge(s_t2, (st - 3) * 16 + (wsel + 1) * 8)
                    # read P[ph%2][p, f = dh*128 + (ys)*8 + x]  (ys = y*2+s)
                    # write S2[p, f = wsel*1024 + (ys)*64 + dh*8 + x]
                    # (y,s) merge into one AP dim on both sides -> 3-dim APs,
                    # one instruction per wsel-half, 8-contig runs both sides
                    in_ap = bass.AP(
                        P[ph % 2],
                        0,
                        [[1024, 128], [8, 16], [128, 8], [1, 8]],  # part, ys, dh, x
                    )
                    out_ap_ = bass.AP(
                        S2[st % 3],
                        wsel * 1024,
                        [[2048, 128], [64, 16], [8, 8], [1, 8]],  # part, ys, dh, x
                    )
                    scalar.activation(
                        out_ap_, in_ap, IDENT_FUNC, bias=bias_sb[:, :], scale=1.0
                    ).then_inc(s_ca, 1)


        @block.vector
        def _(vector):
            for st in range(NS):
                par = st % 2
                if st < NS - 1:
                    vector.wait_ge(s_t2, (st + 1) * 16)
                    if st >= 4:
                        # R[st%4] reusable once out-DMA of strip st-4 done
                        vector.wait_ge(s_od, (st - 3) * 16)
                    # read Q[par][p, f = wsel*1024 + y*128 + w'']
                    # write R[p, f = wsel*1024 + w''*8 + y]
                    in_ap = bass.AP(
                        Q[par],
                        0,
                        [[2048, 128], [1024, 2], [1, 128], [128, 8]],
                    )
                    out_ap_ = bass.AP(
                        R[st % 4],
                        0,
                        [[2048, 128], [1024, 2], [8, 128], [1, 8]],
                    )
                    vector.tensor_copy(out_ap_, in_ap).then_inc(s_cb, 1)
                else:
                    # last strip: split by wsel so the final out-DMA halves
                    # can start as soon as each half is interleaved
                    if st >= 4:
                        vector.wait_ge(s_od, (st - 3) * 16)
                    for wsel in range(2):
                        vector.wait_ge(s_t2, st * 16 + (wsel + 1) * 8)
                        in_ap = bass.AP(
                            Q[par],
                            wsel * 1024,
                            [[2048, 128], [1, 128], [128, 8]],  # part, w'', y
                        )
                        out_ap_ = bass.AP(
                            R[st % 4],
                            wsel * 1024,
                            [[2048, 128], [8, 128], [1, 8]],  # part, w'', y
                        )
                        vector.tensor_copy(out_ap_, in_ap).then_inc(s_cb, 1)

        @block.gpsimd
        def _(pool):
            # out-DMAs on the SWDGE path: keeps the ACT engine free for the
            # PSUM->SBUF copies; descriptors spread via the partition swizzle.
            for st in range(NS - 1):
                pool.wait_ge(s_cb, st + 1)
                pool.dma_start(out_ap(st), R[st % 4][:, :]).then_inc(s_od, 16)
            # last strip in halves (cb increments s_cb twice for it)
            st = NS - 1
            b = (st % N_STRIPS) // STRIPS_PER_BATCH
            stg = (st % N_STRIPS) % STRIPS_PER_BATCH
            base = b * (8 * H * 8 * W) + stg * 128 * 2048
            for wsel in range(2):
                pool.wait_ge(s_cb, NS + wsel)
                pool.dma_start(
                    bass.AP(out, base + wsel * 1024, [[2048, 128], [1, 1024]]),
                    R[st % 4][:, wsel * 1024 : (wsel + 1) * 1024],
                ).then_inc(s_od, 16)

    return nc


def _host_prep(dct: np.ndarray, mean: np.ndarray, std: np.ndarray):
    import ml_dtypes

    dct = np.asarray(dct, dtype=np.float32)
    mean = np.asarray(mean, dtype=np.float64)
    std = np.asarray(std, dtype=np.float64)

    M = _idct_matrix()  # [(x,y), (u,v)]
    bias_vec = (M @ mean + 128.0) / 255.0  # [(x,y)]
    if np.ptp(bias_vec) > 1e-12:
        # General-mean fallback: fold the channel means into the data on the
        # host (never triggers for the spec'd inputs where mean == 0).
        safe_std = np.where(std == 0.0, 1.0, std)
        dct = dct + (mean / safe_std)[None, :, None, None].astype(np.float32)
        bias_scalar = float(128.0 / 255.0)
    else:
        bias_scalar = float(bias_vec[0])

    MT = M.T * std[:, None] / 255.0  # [uv, xy] with xy = x*8+y
    # MTs128[uv*2 + s, y*16 + s*8 + x] = MT[uv, x*8+y]
    MTs = np.zeros((128, 128), dtype=np.float64)
    uv = np.arange(64)
    for s in range(2):
        rows = uv * 2 + s  # partition order (uv, s)
        for y in range(8):
            for x in range(8):
                MTs[rows, y * 16 + s * 8 + x] = MT[uv, x * 8 + y]
    MTs_bf = MTs.astype(ml_dtypes.bfloat16)
    ident_bf = np.eye(128, dtype=np.float64).astype(ml_dtypes.bfloat16)

    dct_bf = dct.astype(ml_dtypes.bfloat16)

    in_maps = []
    for i in range(N_CORES):
        in_maps.append(
            {
                "dct": np.ascontiguousarray(dct_bf[i * B_PC : (i + 1) * B_PC]),
                "mi": np.concatenate([MTs_bf, ident_bf], axis=1),
                "biasv": np.full((128, 1), bias_scalar, dtype=np.float32),
            }
        )
    return in_maps, bias_scalar


def kernel(dct: np.ndarray, mean: np.ndarray, std: np.ndarray) -> np.ndarray:
    from concourse.bass_utils import run_bass_kernel_spmd

    in_maps, bias_scalar = _host_prep(dct, mean, std)
    nc = _build_nc(bias_scalar)

    res = run_bass_kernel_spmd(nc, in_maps, list(range(N_CORES)))

    full = np.empty((B_FULL, 1, 8 * H, 8 * W), dtype=np.float32)
    for i in range(N_CORES):
        full[i * B_PC : (i + 1) * B_PC, 0] = np.asarray(res.results[i]["out"]).astype(np.float32)
    return full
